# revision 51
# baseline (speedup 1.0000x reference)
"""Trainium2 Bass kernel for LocalAttnLayer (sliding-window attention block).

Sharding: 8 cores = (batch b in 0..3) x (sequence half s in 0..1).
Each core processes 2048 tokens with a 128-token halo before its span for
the look-backward attention window. No collectives: the whole layer
(QKV, windowed attention, LN1, FF, LN2) is token-local given the halo.

v3 (software-pipelined + split-fp8 FF1; ~471us vs 900us v1):
  - FF1 runs as split-fp8 DoubleRow: x-hat and W1 each stored as fp8
    hi+lo pairs (packed per-block in one DMA); three passes contract
    256 rows/matmul at 0.5 cyc/row = 25% fewer FF1 PE cycles at ~2^-8
    effective precision (rel err 1.1e-2 vs the 2e-2 gate).
  - Next-chunk K projection units are distributed into the window
    steps (all chunks), keeping them PE-bound so the FF1 savings
    shorten the dense projection block instead of exposing the
    softmax chain.
  - FF2 borrows the idle qk PSUM ring during tail blocks (6-deep ps
    pipeline); identity-LN specialization when ln gammas/betas are
    trivial (generic fallback kept).

v2 (base pipeline, ~474us):
  - Depth-2 chunk pipeline: FF1 of chunk c-1 is interleaved into the
    attention window steps of chunk c, and FF2 of chunk c-1 runs at the
    tail of chunk c, keeping the Tensor engine fed while Activation/DVE
    work through the softmax/LN chains.  Chunk 1's projections are
    pre-emitted into chunk 0's (otherwise filler-less) window steps.
  - K/V are computed only for each chunk's own 512 tokens; the previous
    window's K/V for window 0 are read from the previous chunk's SBUF
    tiles (one-time halo projection for chunk 0).
  - Scores/exp/mask/AV run in pipelined head groups.  Q/K features are
    host-permuted so each score group's matmuls share one PE row base:
    matmuls with different lhsT base partitions into the same PSUM
    tile crash the exec unit (found by hw bisection).
  - Softmax normalization is a per-4-head-group stride-0-broadcast
    multiply on DVE (denominator from the V ones-column), replacing 16
    per-head activation copies per window.
  - FF2 bias matmuls removed: b2 and the ln1_g residual scale are fused
    into a per-token-window xg = xhb*g1 + b2 tile on DVE.
  - Host-side weight relayout so every weight DMA lands 2-8KB
    contiguous runs per partition (fast descriptors).
  - LN rstd computed as exp(-0.5*ln(var+eps)) and all activations
    steered to the one act-func table holding Exp+Ln+Identity+Copy+
    Relu: a single LoadActFuncSet instead of 65 (1.28us each).

Layout strategy per core is otherwise as v1:
  - x^T [1024, 640-span] bf16 feeds all projections.
  - Scores are computed transposed (S^T [ktok, qtok]); softmax
    denominator comes from a ones-column appended to V.
  - Masks are 0/1 multiplies on exp(S).
  - attention scale and q_bias folded into Wq / bq; k_bias dropped;
    v_bias added to the residual input x'; ln1_g folded into ff1_w
    rows; ln1_b folded into ff1 bias and ff2 bias.
"""

import os
import sys

for _p in ("/opt/trn_rl_repo", "/root/.axon_site/_ro/trn_rl_repo"):
    if os.path.isdir(_p) and _p not in sys.path:
        sys.path.insert(0, _p)

import numpy as np
import ml_dtypes

BF16 = ml_dtypes.bfloat16

# Model dims (hardcoded per the problem spec)
B, S, D = 4, 4096, 1024
H, DH = 16, 64
WIN = 128
FH = 2048
LN_EPS = 1e-5

# Per-core sharding
T = 2048          # own tokens per core
HALO = 128
TH = T + HALO     # 2176
NCH = 4           # chunks per core
CT = 512          # tokens per chunk
CW = CT // WIN    # windows per chunk = 4

_PROGRAM_CACHE = {}


def _build_program(simple_ln=False):
    import concourse.bass as bass
    import concourse.tile as tile
    from concourse import bacc, mybir
    from concourse.masks import make_identity
    from contextlib import ExitStack

    f16 = mybir.dt.float16
    f32 = mybir.dt.float32
    f8 = mybir.dt.float8e4
    DR = mybir.MatmulPerfMode.DoubleRow
    AF = mybir.ActivationFunctionType
    ALU = mybir.AluOpType

    nc = bacc.Bacc("TRN2", target_bir_lowering=False, debug=False, num_devices=8)

    # ---- DRAM tensors ----
    xT = nc.dram_tensor("xT", [D, TH], f16, kind="ExternalInput").ap()
    xp = nc.dram_tensor("xp", [T, D], f16, kind="ExternalInput").ap()
    wq = nc.dram_tensor("wq", [8, 128, 8, 128], f16, kind="ExternalInput").ap()
    wk = nc.dram_tensor("wk", [8, 128, 8, 128], f16, kind="ExternalInput").ap()
    wv = nc.dram_tensor("wv", [2, 128, 8, 512], f16, kind="ExternalInput").ap()
    bqd = nc.dram_tensor("bq", [128, 8], f32, kind="ExternalInput").ap()
    w1hl = nc.dram_tensor("w1hl", [16, 128, 16, 128], f8, kind="ExternalInput").ap()
    b1d = nc.dram_tensor("b1", [128, 16], f32, kind="ExternalInput").ap()
    if simple_ln:
        w2hl = nc.dram_tensor(
            "w2hl", [8, 128, 32, 128], f8, kind="ExternalInput").ap()
    else:
        w2 = nc.dram_tensor(
            "w2", [8, 128, 16, 128], f16, kind="ExternalInput").ap()
    g1d = nc.dram_tensor("g1", [D], f16, kind="ExternalInput").ap()
    g2d = nc.dram_tensor("g2", [D], f16, kind="ExternalInput").ap()
    bt2d = nc.dram_tensor("bt2", [D], f16, kind="ExternalInput").ap()
    b2d = nc.dram_tensor("b2", [D], f16, kind="ExternalInput").ap()
    cmkd = nc.dram_tensor("cmk", [WIN, WIN], f16, kind="ExternalInput").ap()
    m0d = nc.dram_tensor("m0", [WIN, WIN], f16, kind="ExternalInput").ap()
    outd = nc.dram_tensor("out", [T, D], f16, kind="ExternalOutput").ap()

    xT_r = xT.rearrange("(dt p) c -> p dt c", p=128)
    xp_r = xp.rearrange("(n p) d -> n p d", p=128)
    out_r = outd.rearrange("(n p) d -> n p d", p=128)

    def bcast_ap(src_ap, parts=128):
        return bass.AP(
            tensor=src_ap.tensor,
            offset=src_ap.offset,
            ap=[[0, parts]] + [list(x) for x in src_ap.ap],
        )

    with tile.TileContext(nc) as tc, ExitStack() as ctx:
        # ---- pools ----
        singles = ctx.enter_context(tc.tile_pool(name="singles", bufs=1))
        xt_pool = ctx.enter_context(tc.tile_pool(name="xt", bufs=2))
        kt_pool = ctx.enter_context(tc.tile_pool(name="kt", bufs=3))
        v_pool = ctx.enter_context(tc.tile_pool(name="v", bufs=2))
        qt_pool = ctx.enter_context(tc.tile_pool(name="qt", bufs=1))
        es_pool = ctx.enter_context(tc.tile_pool(name="es", bufs=2))
        at_pool = ctx.enter_context(tc.tile_pool(name="at", bufs=2))
        xp_pool = ctx.enter_context(tc.tile_pool(name="xpp", bufs=2))
        xhb_pool = ctx.enter_context(tc.tile_pool(name="xhb", bufs=5))
        xht_pool = ctx.enter_context(tc.tile_pool(name="xht", bufs=2))
        xhtl_pool = ctx.enter_context(tc.tile_pool(name="xhtl", bufs=2))
        ht_pool = ctx.enter_context(tc.tile_pool(name="ht", bufs=1))
        p2_pool = ctx.enter_context(tc.tile_pool(name="p2", bufs=4))
        xg_pool = ctx.enter_context(tc.tile_pool(name="xg", bufs=4))
        oh_pool = ctx.enter_context(tc.tile_pool(name="oh", bufs=2))
        wblk_pool = ctx.enter_context(tc.tile_pool(name="wblk", bufs=3))
        wv_pool = ctx.enter_context(tc.tile_pool(name="wv", bufs=1))
        w2_pool = ctx.enter_context(tc.tile_pool(name="w2", bufs=3))
        small = ctx.enter_context(tc.tile_pool(name="small", bufs=6))

        proj_ps = ctx.enter_context(tc.tile_pool(name="projps", bufs=3, space="PSUM"))
        qk_ps = ctx.enter_context(tc.tile_pool(name="qkps", bufs=3, space="PSUM"))
        avtp_ps = ctx.enter_context(tc.tile_pool(name="avtp", bufs=2, space="PSUM"))

        # ---- startup-critical DMAs first: first Q weight block + x slab ----
        wqb_first = wblk_pool.tile([128, 8, 128], f16, tag="wblk")
        nc.sync.dma_start(out=wqb_first, in_=wq[0])
        xt_first = xt_pool.tile([128, 8, CT], f16, tag="xt")
        nc.sync.dma_start(out=xt_first[:, 0:2, :], in_=xT_r[:, 0:2, HALO:HALO + CT])
        bq_sb = singles.tile([128, 8], f32)
        nc.sync.dma_start(out=bq_sb, in_=bqd)
        nc.sync.dma_start(out=xt_first[:, 2:8, :], in_=xT_r[:, 2:8, HALO:HALO + CT])
        xthalo = singles.tile([128, 8, HALO], f16)
        nc.sync.dma_start(out=xthalo, in_=xT_r[:, :, 0:HALO])

        # ---- constants: tiles now, DMAs deferred past chunk-0 weight loads
        ident = singles.tile([128, 128], f16)
        make_identity(nc, ident)
        cm = singles.tile([WIN, WIN], f16)
        m0t = singles.tile([WIN, WIN], f16)
        g1b = singles.tile([128, D], f16)
        g2b = singles.tile([128, D], f16)
        bt2b = singles.tile([128, D], f16)
        b2b = singles.tile([128, D], f16)
        b1_sb = singles.tile([128, 16], f32)

        def load_constants():
            nc.sync.dma_start(out=cm, in_=cmkd)
            nc.sync.dma_start(out=m0t, in_=m0d)
            nc.sync.dma_start(out=g1b, in_=bcast_ap(g1d))
            nc.sync.dma_start(out=g2b, in_=bcast_ap(g2d))
            nc.sync.dma_start(out=bt2b, in_=bcast_ap(bt2d))
            nc.sync.dma_start(out=b2b, in_=bcast_ap(b2d))
            nc.sync.dma_start(out=b1_sb, in_=b1d)

        epst = singles.tile([128, 1], f32)
        nc.vector.memset(epst, LN_EPS)
        ebt = singles.tile([128, 1], f32)
        nc.vector.memset(ebt, -6.931471805599453)
        kthalo = singles.tile([128, 8, 128], f16)
        v0_t = singles.tile([128, H, DH + 1], f16)

        cm_b = cm[:, :].unsqueeze(1).broadcast_to([128, 2, WIN])
        m0_b = m0t[:, :].unsqueeze(1).broadcast_to([128, 2, WIN])

        # ---- per-chunk state carried across the pipeline ----
        kt_prev = None       # kt slab of previous chunk
        v_prev = None        # v slab of previous chunk
        ht_prev = None       # FF1 output of previous chunk (filled this chunk)
        xht_prev = None      # x-hat^T of previous chunk (fp8 hi)
        xhtl_prev = None     # x-hat^T fp8 lo residual
        xhb_prev = None      # per-window xhb list of previous chunk
        xt_next = None       # prefetched xT slab

        def emit_qproj_unit(xt_t, qt_t, qc, wqb=None):
            if wqb is None:
                wqb = wblk_pool.tile([128, 8, 128], f16, tag="wblk", name="wqb")
                nc.sync.dma_start(out=wqb, in_=wq[qc])
            ps = proj_ps.tile([128, 512], f32, tag="pp")
            for d in range(8):
                nc.tensor.matmul(
                    ps, lhsT=wqb[:, d, :], rhs=xt_t[:, d, 0:CT],
                    start=(d == 0), stop=(d == 7),
                )
            nc.scalar.activation(
                qt_t[:, qc, :], ps, AF.Identity,
                bias=bq_sb[:, qc:qc + 1], scale=1.0,
            )

        def emit_kproj_unit(xt_t, kt_t, kc, first):
            wkb = wblk_pool.tile([128, 8, 128], f16, tag="wblk", name="wkb")
            nc.sync.dma_start(out=wkb, in_=wk[kc])
            ps = proj_ps.tile([128, 512], f32, tag="pp")
            for d in range(8):
                nc.tensor.matmul(
                    ps, lhsT=wkb[:, d, :], rhs=xt_t[:, d, 0:CT],
                    start=(d == 0), stop=(d == 7),
                )
            nc.scalar.activation(kt_t[:, kc, :], ps, AF.Copy, scale=1.0)
            if first:
                psh = proj_ps.tile([128, 512], f32, tag="pp")
                for d in range(8):
                    nc.tensor.matmul(
                        psh[:, 0:128], lhsT=wkb[:, d, :], rhs=xthalo[:, d, :],
                        start=(d == 0), stop=(d == 7),
                    )
                nc.scalar.activation(
                    kthalo[:, kc, :], psh[:, 0:128], AF.Copy, scale=1.0
                )

        def emit_vproj_half(xt_t, v_t, vc2, first):
            wvb = wv_pool.tile([128, 8, 512], f16, tag="wv", name="wvb")
            nc.sync.dma_start(out=wvb, in_=wv[vc2])
            vts = range(-1, 4) if first else range(0, 4)
            for vt in vts:
                ps = proj_ps.tile([128, 512], f32, tag="pp")
                for d in range(8):
                    xsl = (xthalo[:, d, :] if vt < 0
                           else xt_t[:, d, vt * 128:(vt + 1) * 128])
                    nc.tensor.matmul(
                        ps, lhsT=xsl, rhs=wvb[:, d, :],
                        start=(d == 0), stop=(d == 7),
                    )
                psv = ps.rearrange("p (h e) -> p h e", e=DH)
                if vt < 0:
                    nc.scalar.activation(
                        v0_t[:, vc2 * 8:(vc2 + 1) * 8, 0:DH], psv,
                        AF.Copy, scale=1.0,
                    )
                else:
                    nc.scalar.activation(
                        v_t[:, vt, vc2 * 8:(vc2 + 1) * 8, 0:DH], psv,
                        AF.Copy, scale=1.0,
                    )

        def emit_proj(chn, xt_t, wqb0=None, kt_pre=None, v_pre=None):
            """Q/K/V projections for chunk chn. Returns (qt, kt, v)."""
            first = chn == 0
            qt_t = qt_pool.tile([128, 8, CT], f16, tag="qt")
            for qc in range(8):
                emit_qproj_unit(xt_t, qt_t, qc, wqb=wqb0 if qc == 0 else None)
            if kt_pre is not None:
                kt_t = kt_pre
            else:
                kt_t = kt_pool.tile([128, 8, CT], f16, tag="kt")
                for kc in range(8):
                    emit_kproj_unit(xt_t, kt_t, kc, first)
            if v_pre is not None:
                return qt_t, kt_t, v_pre
            v_t = v_pool.tile([128, 4, H, DH + 1], f16, tag="v")
            nc.vector.memset(v_t[:, :, :, DH:DH + 1], 1.0)
            if first:
                nc.vector.memset(v0_t[:, :, DH:DH + 1], 1.0)
            for vc2 in range(2):
                emit_vproj_half(xt_t, v_t, vc2, first)
            return qt_t, kt_t, v_t

        def kt_tile(chn, kt_t, j, h):
            """lhsT AP for k-tile j (0..4) of chunk chn, head h.

            Q/K features are host-permuted so head h sits at partition rows
            (h//8)*64.. and column tile h%8 — score groups of 4 consecutive
            heads then share one PE row base (mixed row bases into one PSUM
            tile crash the exec unit)."""
            r0 = (h // 8) * 64
            if j == 0:
                if chn == 0:
                    return kthalo[r0:r0 + 64, h % 8, :]
                return kt_prev[r0:r0 + 64, h % 8, 384:512]
            return kt_t[r0:r0 + 64, h % 8, (j - 1) * 128:j * 128]

        def v_tile(chn, v_t, j, h):
            if j == 0:
                if chn == 0:
                    return v0_t[:, h, :]
                return v_prev[:, 3, h, :]
            return v_t[:, j - 1, h, :]

        def emit_ff1_quarter(q):
            """FF1 for hc in [4q, 4q+4): split-fp8 DoubleRow (hi*hi +
            hi*lo + lo*hi), 25% fewer PE cycles than bf16."""
            for hc in range(4 * q, 4 * q + 4):
                w1b = wblk_pool.tile([128, 16, 128], f8, tag="wblk", name="w1b")
                nc.sync.dma_start(out=w1b, in_=w1hl[hc])
                ps = proj_ps.tile([128, 512], f32, tag="pp")
                n = 0
                for (lo, R) in ((0, xht_prev), (0, xhtl_prev),
                                (8, xht_prev)):
                    for i in range(4):
                        nc.tensor.matmul(
                            ps, lhsT=w1b[:, lo + 2 * i:lo + 2 * i + 2, :],
                            rhs=R[:, 2 * i:2 * i + 2, :],
                            start=(n == 0), stop=(n == 11), perf_mode=DR,
                        )
                        n += 1
                if simple_ln:
                    nc.scalar.activation(ht_prev[:, hc, :], ps, AF.Relu,
                                         scale=1.0)
                    nc.vector.scalar_tensor_tensor(
                        htl_prev[:, hc, :], ps, 0.0, ht_prev[:, hc, :],
                        op0=ALU.max, op1=ALU.subtract,
                    )
                else:
                    nc.scalar.activation(
                        ht_prev[:, hc, :], ps, AF.Relu,
                        bias=b1_sb[:, hc:hc + 1], scale=1.0,
                    )

        def make_xg(xhb_t):
            xg_t = xg_pool.tile([128, D], f16, tag="xg", name="xg_t")
            if simple_ln:
                nc.vector.tensor_add(xg_t, xhb_t, b2b)
            else:
                nc.vector.tensor_mul(xg_t, xhb_t, g1b)
                nc.vector.tensor_add(xg_t, xg_t, b2b)
            return xg_t

        def emit_ff2(wg_base, xg_list, last=False):
            """FF2 + LN2 + output for the previous chunk (its global window
            base is wg_base)."""
            p2_list = [
                p2_pool.tile([128, D], f32, tag="p2", name=f"p2_{t}")
                for t in range(CW)
            ]
            st2_list = [
                small.tile([128, 8, 6], f32, tag="st2", name=f"st2_{t}")
                for t in range(CW)
            ]
            def emit_ln2(t):
                mv2 = small.tile([128, 2], f32, tag="mv2")
                nc.vector.bn_aggr(out=mv2, in_=st2_list[t])
                lv2 = small.tile([128, 1], f32, tag="sd2")
                nc.scalar.activation(lv2, mv2[:, 1:2], AF.Ln, bias=epst)
                rstd2 = small.tile([128, 1], f32, tag="rstd2")
                nc.scalar.activation(rstd2, lv2, AF.Exp, scale=-0.5)
                nmr2 = small.tile([128, 1], f32, tag="nmr2")
                nc.vector.tensor_scalar(
                    nmr2, mv2[:, 0:1], rstd2, -1.0, op0=ALU.mult, op1=ALU.mult
                )
                oh = oh_pool.tile([128, D], f16, tag="oh", name="oh")
                for yc in range(2):
                    sl2 = slice(yc * 512, (yc + 1) * 512)
                    nc.scalar.activation(
                        oh[:, sl2], p2_list[t][:, sl2], AF.Identity,
                        bias=nmr2, scale=rstd2,
                    )
                    if not simple_ln:
                        nc.vector.tensor_mul(
                            oh[:, sl2], oh[:, sl2], g2b[:, sl2])
                        nc.vector.tensor_add(
                            oh[:, sl2], oh[:, sl2], bt2b[:, sl2])
                    nc.sync.dma_start(
                        out=out_r[wg_base + t][:, sl2], in_=oh[:, sl2]
                    )

            for y8 in range(8):
                if simple_ln:
                    w2b = w2_pool.tile([128, 32, 128], f8, tag="w2")
                    nc.sync.dma_start(out=w2b, in_=w2hl[y8])
                else:
                    w2b = w2_pool.tile([128, 16, 128], f16, tag="w2")
                    nc.sync.dma_start(out=w2b, in_=w2[y8])
                psp = qk_ps if y8 % 2 else proj_ps
                ps = psp.tile([128, 512], f32, tag="qk" if y8 % 2 else "pp")
                for t in range(CW):
                    sl = slice(t * 128, (t + 1) * 128)
                    if simple_ln:
                        n = 0
                        for (L, ro) in ((ht_prev, 0), (ht_prev, 16),
                                        (htl_prev, 0)):
                            for i in range(8):
                                nc.tensor.matmul(
                                    ps[:, sl],
                                    lhsT=L[:, 2 * i:2 * i + 2, sl],
                                    rhs=w2b[:, ro + 2 * i:ro + 2 * i + 2, :],
                                    start=(n == 0), stop=(n == 23),
                                    perf_mode=DR,
                                )
                                n += 1
                    else:
                        for hc in range(16):
                            nc.tensor.matmul(
                                ps[:, sl], lhsT=ht_prev[:, hc, sl],
                                rhs=w2b[:, hc, :],
                                start=(hc == 0), stop=(hc == 15),
                            )
                    ysl = slice(y8 * 128, (y8 + 1) * 128)
                    nc.vector.tensor_add(
                        p2_list[t][:, ysl], ps[:, sl], xg_list[t][:, ysl]
                    )
                    nc.vector.bn_stats(
                        out=st2_list[t][:, y8, :], in_=p2_list[t][:, ysl]
                    )
                    if y8 == 7:
                        emit_ln2(t)

        def emit_tp(w, xhb_w, xht_t, xhtl_t, on_act=False):
            """Transpose window w's x-hat into fp8 hi/lo xht slabs."""
            tp = avtp_ps.tile([128, 8, 128], f16, tag="avtp")
            for dt in range(8):
                nc.tensor.transpose(
                    tp[:, dt, :], xhb_w[:, dt * 128:(dt + 1) * 128], ident
                )
            wsl = slice(w * 128, (w + 1) * 128)
            nc.scalar.activation(xht_t[:, :, wsl], tp, AF.Copy, scale=1.0)
            nc.vector.tensor_sub(xhtl_t[:, :, wsl], tp, xht_t[:, :, wsl])

        xt_next = xt_first

        next_proj = None     # (qt, kt, v) pre-emitted for this chunk
        kt_next = None       # K slab of the next chunk, filled in our windows
        for chn in range(NCH):
            xt_t = xt_next
            if next_proj is not None:
                qt_t, kt_t, v_t = next_proj
                next_proj = None
            else:
                qt_t, kt_t, v_t = emit_proj(
                    chn, xt_t,
                    wqb0=wqb_first if chn == 0 else wqb_next,
                    kt_pre=kt_next)
            if 0 < chn < NCH - 1:
                kt_next = kt_pool.tile([128, 8, CT], f16, tag="kt",
                                       name="kt_next")
            else:
                kt_next = None
            if chn == 0:
                load_constants()
            if chn + 1 < NCH:
                xt_next = xt_pool.tile([128, 8, CT], f16, tag="xt")
                c0 = (chn + 1) * CT + HALO
                nc.sync.dma_start(out=xt_next, in_=xT_r[:, :, c0:c0 + CT])
            if chn == 0:
                kt1 = kt_pool.tile([128, 8, CT], f16, tag="kt")
                qt1 = qt_pool.tile([128, 8, CT], f16, tag="qt")

            xht_t = xht_pool.tile([128, 8, CT], f8, tag="xht")
            xhtl_t = xhtl_pool.tile([128, 8, CT], f8, tag="xhtl")
            xhb_list = []
            xg_cur = []
            if chn > 0:
                if simple_ln:
                    ht_prev = ht_pool.tile([128, 16, CT], f8, tag="ht")
                    htl_prev = ht_pool.tile([128, 16, CT], f8, tag="htl")
                else:
                    ht_prev = ht_pool.tile([128, 16, CT], f16, tag="ht")

            for w in range(CW):
                wg = chn * CW + w
                # residual slab for this window (early DMA)
                xp_t = xp_pool.tile([128, D], f16, tag="xp")
                nc.sync.dma_start(out=xp_t, in_=xp_r[wg])

                # ---- scores: QK^T, exp + mask in 4-head groups (pipelined)
                es = es_pool.tile([128, H, 2 * WIN], f16, tag="es")
                for g in range(8):
                    st = qk_ps.tile([128, 2, 256], f32, tag="qk")
                    for hh in range(2):
                        h = g * 2 + hh
                        r0 = (h // 8) * 64
                        for half in range(2):
                            j = w + half
                            nc.tensor.matmul(
                                st[:, hh, half * 128:(half + 1) * 128],
                                lhsT=kt_tile(chn, kt_t, j, h),
                                rhs=qt_t[r0:r0 + 64, h % 8, w * 128:(w + 1) * 128],
                                start=True, stop=True,
                            )
                    gs = slice(g * 2, (g + 1) * 2)
                    nc.scalar.activation(es[:, gs, :], st, AF.Exp, bias=ebt)
                    nc.vector.tensor_mul(
                        es[:, gs, WIN:2 * WIN], es[:, gs, WIN:2 * WIN], cm_b
                    )
                    if wg == 0:
                        nc.vector.tensor_mul(
                            es[:, gs, 0:WIN], es[:, gs, 0:WIN], m0_b
                        )

                # ---- PE fillers while softmax chain runs
                if w > 0:
                    emit_tp(w - 1, xhb_list[w - 1], xht_t, xhtl_t)
                if chn > 0:
                    emit_ff1_quarter(w)
                    if kt_next is not None:
                        emit_kproj_unit(xt_next, kt_next, 2 * w, False)
                        emit_kproj_unit(xt_next, kt_next, 2 * w + 1, False)
                else:
                    emit_kproj_unit(xt_next, kt1, 2 * w, False)
                    emit_kproj_unit(xt_next, kt1, 2 * w + 1, False)

                # ---- AV in 4-head groups with ones-column denominators
                at_t = at_pool.tile([128, D], f32, tag="at")
                at_r = at_t.rearrange("p (h e) -> p h e", e=DH)
                rden = small.tile([128, H], f32, tag="rden")
                for g in range(4):
                    av = avtp_ps.tile([128, 512], f32, tag="avtp")
                    av_r = av.rearrange("p (h x) -> p h x", x=128)
                    for hh in range(4):
                        h = g * 4 + hh
                        for half in range(2):
                            j = w + half
                            nc.tensor.matmul(
                                av[:, hh * 128:hh * 128 + DH + 1],
                                lhsT=es[:, h, half * 128:(half + 1) * 128],
                                rhs=v_tile(chn, v_t, j, h),
                                start=(half == 0), stop=(half == 1),
                            )
                    nc.vector.reciprocal(
                        rden[:, g * 4:(g + 1) * 4], av_r[:, :, DH:DH + 1]
                    )
                    nc.vector.tensor_mul(
                        at_r[:, g * 4:(g + 1) * 4, :],
                        av_r[:, :, 0:DH],
                        rden[:, g * 4:(g + 1) * 4].unsqueeze(2).broadcast_to(
                            [128, 4, DH]
                        ),
                    )

                # ---- residual + LN1
                nc.vector.tensor_add(at_t, at_t, xp_t)
                stats = small.tile([128, 2, 6], f32, tag="stats")
                pre1v = at_t.rearrange("p (a b) -> p a b", b=512)
                for sg in range(2):
                    nc.vector.bn_stats(out=stats[:, sg, :], in_=pre1v[:, sg, :])
                mv = small.tile([128, 2], f32, tag="mv")
                nc.vector.bn_aggr(out=mv, in_=stats)
                lv = small.tile([128, 1], f32, tag="sd")
                nc.scalar.activation(lv, mv[:, 1:2], AF.Ln, bias=epst)
                rstd = small.tile([128, 1], f32, tag="rstd")
                nc.scalar.activation(rstd, lv, AF.Exp, scale=-0.5)
                nmr = small.tile([128, 1], f32, tag="nmr")
                nc.vector.tensor_scalar(
                    nmr, mv[:, 0:1], rstd, -1.0, op0=ALU.mult, op1=ALU.mult
                )
                xhb = xhb_pool.tile([128, D], f16, tag="xhb")
                nc.scalar.activation(xhb, at_t, AF.Identity, bias=nmr, scale=rstd)
                xhb_list.append(xhb)
                if chn > 0:
                    xg_cur.append(make_xg(xhb_prev[w]))

            # ---- chunk tail: last transpose, then FF2 of previous chunk
            if chn == 0:
                emit_qproj_unit(xt_next, qt1, 0)
                emit_qproj_unit(xt_next, qt1, 1)
                emit_tp(CW - 1, xhb_list[CW - 1], xht_t, xhtl_t, on_act=True)
                for qc in range(2, 8):
                    emit_qproj_unit(xt_next, qt1, qc)
                v1 = v_pool.tile([128, 4, H, DH + 1], f16, tag="v")
                nc.vector.memset(v1[:, :, :, DH:DH + 1], 1.0)
                for vc2 in range(2):
                    emit_vproj_half(xt_next, v1, vc2, False)
                next_proj = (qt1, kt1, v1)
            else:
                emit_tp(CW - 1, xhb_list[CW - 1], xht_t, xhtl_t, on_act=True)
            if chn > 0:
                if chn + 1 < NCH:
                    wqb_next = wblk_pool.tile(
                        [128, 8, 128], f16, tag="wblk", name="wqb_next")
                    nc.sync.dma_start(out=wqb_next, in_=wq[0])
                emit_ff2((chn - 1) * CW, xg_cur)

            kt_prev, v_prev = kt_t, v_t
            xht_prev, xhtl_prev = xht_t, xhtl_t
            xhb_prev = xhb_list

        # ---- epilogue: FF1 + FF2 of the last chunk ----
        if simple_ln:
            ht_prev = ht_pool.tile([128, 16, CT], f8, tag="ht")
            htl_prev = ht_pool.tile([128, 16, CT], f8, tag="htl")
        else:
            ht_prev = ht_pool.tile([128, 16, CT], f16, tag="ht")
        xg_last = [make_xg(xhb_prev[t]) for t in range(CW)]
        for q in range(CW):
            emit_ff1_quarter(q)
        emit_ff2((NCH - 1) * CW, xg_last, last=True)

    # Steer every activation to the one table containing Exp+Ln+Identity+
    # Copy+Relu ('natural_log_exp_and_others') so a single LoadActFuncSet
    # suffices; the greedy placement otherwise alternates exp/ln tables at
    # 1.28us per reload. Names and dict order (= act_func_set_id) are kept.
    import concourse.bacc as bacc_mod
    orig_tables = bacc_mod.get_activation_tables
    target = "natural_log_exp_and_others"
    mine = {AF.Exp, AF.Ln, AF.Identity, AF.Copy, AF.Relu}

    def steered(arch):
        tabs = orig_tables(arch)
        return {
            name: (set(s) if name == target else set(s) - mine)
            for name, s in tabs.items()
        }

    bacc_mod.get_activation_tables = steered
    try:
        nc.compile()
    finally:
        bacc_mod.get_activation_tables = orig_tables
    return nc


def _is_fast(ln1_g, ln2_g, ln2_b, ff1_b, ln1_b):
    return (np.allclose(np.asarray(ln2_g, np.float32), 1.0)
            and np.allclose(np.asarray(ln2_b, np.float32), 0.0)
            and np.allclose(np.asarray(ln1_g, np.float32), 1.0)
            and np.allclose(np.asarray(ff1_b, np.float32), 0.0)
            and np.allclose(np.asarray(ln1_b, np.float32), 0.0))


def _get_program(simple_ln=False):
    key = ("nc", simple_ln)
    if key not in _PROGRAM_CACHE:
        _PROGRAM_CACHE[key] = _build_program(simple_ln=simple_ln)
    return _PROGRAM_CACHE[key]


def make_in_maps(x, q_proj, k_proj, v_proj, q_bias, k_bias, v_bias,
                 ln1_g, ln1_b, ln2_g, ln2_b, ff1_w, ff1_b, ff2_w, ff2_b):
    """Host-side prep: fold biases/scales, relayout weights, shard."""
    x = np.asarray(x, np.float32)
    scale = DH ** -0.5

    Wq = (np.transpose(np.asarray(q_proj, np.float32), (1, 0, 2)).reshape(D, D)
          * scale)
    Wk = np.transpose(np.asarray(k_proj, np.float32), (1, 0, 2)).reshape(D, D)
    Wv = np.transpose(np.asarray(v_proj, np.float32), (1, 0, 2)).reshape(D, D)
    bq_full = (np.asarray(q_bias, np.float32).reshape(D) * scale)
    bv_full = np.asarray(v_bias, np.float32).reshape(D)

    ln1_g = np.asarray(ln1_g, np.float32)
    ln1_b = np.asarray(ln1_b, np.float32)
    ff1_w = np.asarray(ff1_w, np.float32)
    ff1_b = np.asarray(ff1_b, np.float32)
    ff2_w = np.asarray(ff2_w, np.float32)
    ff2_b = np.asarray(ff2_b, np.float32)

    W1 = ff1_w * ln1_g[:, None]                 # fold ln1_g into rows
    b1_full = ff1_b + ln1_b @ ff1_w             # fold ln1_b into ff1 bias
    b2_full = ff2_b + ln1_b                     # fold ln1_b into ff2 bias

    # Q/K head permutation: head h -> partition rows (h//8)*64, col tile h%8,
    # so 4-consecutive-head score groups share one PE row base.
    qk_perm = np.empty(D, np.int64)
    for h in range(H):
        e = np.arange(DH)
        qk_perm[(h % 8) * 128 + (h // 8) * 64 + e] = h * DH + e
    Wq = Wq[:, qk_perm]
    Wk = Wk[:, qk_perm]
    bq_full = bq_full[qk_perm]

    def relayout(W, n_out_blk, blk):
        # [K, N] -> [nb, 128, K//128, blk] so each SBUF partition's data is
        # one contiguous run per DMA block.
        K, N = W.shape
        a = W.reshape(K // 128, 128, n_out_blk, blk).transpose(2, 1, 0, 3)
        return np.ascontiguousarray(a).astype(np.float16)

    wq_l = relayout(Wq, 8, 128)
    wk_l = relayout(Wk, 8, 128)
    wv_l = relayout(Wv, 2, 512)
    E4 = ml_dtypes.float8_e4m3

    def relayout8(W, n_out_blk, blk):
        K, N = W.shape
        a = W.reshape(K // 128, 128, n_out_blk, blk).transpose(2, 1, 0, 3)
        return np.ascontiguousarray(a).astype(E4)

    W1h = W1.astype(E4).astype(np.float32)
    w1hl_l = np.concatenate(
        [relayout8(W1h, 16, 128), relayout8(W1 - W1h, 16, 128)], axis=2)
    if _is_fast(ln1_g, ln2_g, ln2_b, ff1_b, ln1_b):
        W2h = ff2_w.astype(E4).astype(np.float32)
        w2_kv = {"w2hl": np.concatenate(
            [relayout8(W2h, 8, 128), relayout8(ff2_w - W2h, 8, 128)], axis=2)}
    else:
        w2_kv = {"w2": relayout(ff2_w, 8, 128)}

    bq_l = np.ascontiguousarray(bq_full.reshape(8, 128).T.astype(np.float32))
    b1_l = np.ascontiguousarray(b1_full.reshape(16, 128).T.astype(np.float32))

    kq = np.arange(WIN)
    cmk = (kq[None, :] >= kq[:, None]).astype(np.float16)  # [k, q]

    common = {
        "wq": wq_l, "wk": wk_l, "wv": wv_l,
        "bq": bq_l, "w1hl": w1hl_l, "b1": b1_l, **w2_kv,
        "b2": b2_full.astype(np.float16),
        "g1": ln1_g.astype(np.float16),
        "g2": np.asarray(ln2_g, np.float16),
        "bt2": np.asarray(ln2_b, np.float16),
        "cmk": cmk,
    }

    in_maps = []
    for b in range(B):
        for s in range(2):
            own = x[b, s * T:(s + 1) * T]
            if s == 0:
                halo = np.zeros((HALO, D), np.float32)
                m0 = np.zeros((WIN, WIN), np.float16)
            else:
                halo = x[b, s * T - HALO:s * T]
                m0 = np.ones((WIN, WIN), np.float16)
            xta = np.ascontiguousarray(
                np.concatenate([halo, own], axis=0).T).astype(np.float16)
            xpa = own + bv_full[None, :]
            in_maps.append({
                **common,
                "xT": xta,
                "xp": np.ascontiguousarray(xpa).astype(np.float16),
                "m0": m0,
            })
    return in_maps


def gather_outputs(results):
    out = np.empty((B, S, D), np.float32)
    for b in range(B):
        for s in range(2):
            out[b, s * T:(s + 1) * T] = np.asarray(
                results[b * 2 + s]["out"], dtype=np.float32)
    return out


def kernel(**inputs):
    from concourse import bass_utils

    simple_ln = _is_fast(inputs["ln1_g"], inputs["ln2_g"],
                         inputs["ln2_b"], inputs["ff1_b"], inputs["ln1_b"])
    nc = _get_program(simple_ln=simple_ln)
    in_maps = make_in_maps(**inputs)
    res = bass_utils.run_bass_kernel_spmd(nc, in_maps, core_ids=list(range(8)))
    return gather_outputs(res.results)


# revision 52
# speedup vs baseline: 1.0110x; 1.0110x over previous
"""Trainium2 Bass kernel for LocalAttnLayer (sliding-window attention block).

Sharding: 8 cores = (batch b in 0..3) x (sequence half s in 0..1).
Each core processes 2048 tokens with a 128-token halo before its span for
the look-backward attention window. No collectives: the whole layer
(QKV, windowed attention, LN1, FF, LN2) is token-local given the halo.

v3 (software-pipelined + split-fp8 FF1; ~471us vs 900us v1):
  - FF1 runs as split-fp8 DoubleRow: x-hat and W1 each stored as fp8
    hi+lo pairs (packed per-block in one DMA); three passes contract
    256 rows/matmul at 0.5 cyc/row = 25% fewer FF1 PE cycles at ~2^-8
    effective precision (rel err 1.1e-2 vs the 2e-2 gate).
  - Next-chunk K projection units are distributed into the window
    steps (all chunks), keeping them PE-bound so the FF1 savings
    shorten the dense projection block instead of exposing the
    softmax chain.
  - FF2 borrows the idle qk PSUM ring during tail blocks (6-deep ps
    pipeline); identity-LN specialization when ln gammas/betas are
    trivial (generic fallback kept).

v2 (base pipeline, ~474us):
  - Depth-2 chunk pipeline: FF1 of chunk c-1 is interleaved into the
    attention window steps of chunk c, and FF2 of chunk c-1 runs at the
    tail of chunk c, keeping the Tensor engine fed while Activation/DVE
    work through the softmax/LN chains.  Chunk 1's projections are
    pre-emitted into chunk 0's (otherwise filler-less) window steps.
  - K/V are computed only for each chunk's own 512 tokens; the previous
    window's K/V for window 0 are read from the previous chunk's SBUF
    tiles (one-time halo projection for chunk 0).
  - Scores/exp/mask/AV run in pipelined head groups.  Q/K features are
    host-permuted so each score group's matmuls share one PE row base:
    matmuls with different lhsT base partitions into the same PSUM
    tile crash the exec unit (found by hw bisection).
  - Softmax normalization is a per-4-head-group stride-0-broadcast
    multiply on DVE (denominator from the V ones-column), replacing 16
    per-head activation copies per window.
  - FF2 bias matmuls removed: b2 and the ln1_g residual scale are fused
    into a per-token-window xg = xhb*g1 + b2 tile on DVE.
  - Host-side weight relayout so every weight DMA lands 2-8KB
    contiguous runs per partition (fast descriptors).
  - LN rstd computed as exp(-0.5*ln(var+eps)) and all activations
    steered to the one act-func table holding Exp+Ln+Identity+Copy+
    Relu: a single LoadActFuncSet instead of 65 (1.28us each).

Layout strategy per core is otherwise as v1:
  - x^T [1024, 640-span] bf16 feeds all projections.
  - Scores are computed transposed (S^T [ktok, qtok]); softmax
    denominator comes from a ones-column appended to V.
  - Masks are 0/1 multiplies on exp(S).
  - attention scale and q_bias folded into Wq / bq; k_bias dropped;
    v_bias added to the residual input x'; ln1_g folded into ff1_w
    rows; ln1_b folded into ff1 bias and ff2 bias.
"""

import os
import sys

for _p in ("/opt/trn_rl_repo", "/root/.axon_site/_ro/trn_rl_repo"):
    if os.path.isdir(_p) and _p not in sys.path:
        sys.path.insert(0, _p)

import numpy as np
import ml_dtypes

BF16 = ml_dtypes.bfloat16

# Model dims (hardcoded per the problem spec)
B, S, D = 4, 4096, 1024
H, DH = 16, 64
WIN = 128
FH = 2048
LN_EPS = 1e-5

# Per-core sharding
T = 2048          # own tokens per core
HALO = 128
TH = T + HALO     # 2176
NCH = 4           # chunks per core
CT = 512          # tokens per chunk
CW = CT // WIN    # windows per chunk = 4

_PROGRAM_CACHE = {}


def _build_program(simple_ln=False):
    import concourse.bass as bass
    import concourse.tile as tile
    from concourse import bacc, mybir
    from concourse.masks import make_identity
    from contextlib import ExitStack

    f16 = mybir.dt.float16
    f32 = mybir.dt.float32
    f8 = mybir.dt.float8e4
    DR = mybir.MatmulPerfMode.DoubleRow
    AF = mybir.ActivationFunctionType
    ALU = mybir.AluOpType

    nc = bacc.Bacc("TRN2", target_bir_lowering=False, debug=False, num_devices=8)

    # ---- DRAM tensors ----
    xT = nc.dram_tensor("xT", [D, TH], f16, kind="ExternalInput").ap()
    xp = nc.dram_tensor("xp", [T, D], f16, kind="ExternalInput").ap()
    wq = nc.dram_tensor("wq", [8, 128, 8, 128], f16, kind="ExternalInput").ap()
    wk = nc.dram_tensor("wk", [8, 128, 8, 128], f16, kind="ExternalInput").ap()
    wv = nc.dram_tensor("wv", [2, 128, 8, 512], f16, kind="ExternalInput").ap()
    bqd = nc.dram_tensor("bq", [128, 8], f32, kind="ExternalInput").ap()
    w1hl = nc.dram_tensor("w1hl", [16, 128, 16, 128], f8, kind="ExternalInput").ap()
    b1d = nc.dram_tensor("b1", [128, 16], f32, kind="ExternalInput").ap()
    if simple_ln:
        w2hl = nc.dram_tensor(
            "w2hl", [8, 128, 32, 128], f8, kind="ExternalInput").ap()
    else:
        w2 = nc.dram_tensor(
            "w2", [8, 128, 16, 128], f16, kind="ExternalInput").ap()
    g1d = nc.dram_tensor("g1", [D], f16, kind="ExternalInput").ap()
    g2d = nc.dram_tensor("g2", [D], f16, kind="ExternalInput").ap()
    bt2d = nc.dram_tensor("bt2", [D], f16, kind="ExternalInput").ap()
    b2d = nc.dram_tensor("b2", [D], f16, kind="ExternalInput").ap()
    cmkd = nc.dram_tensor("cmk", [WIN, WIN], f16, kind="ExternalInput").ap()
    m0d = nc.dram_tensor("m0", [WIN, WIN], f16, kind="ExternalInput").ap()
    outd = nc.dram_tensor("out", [T, D], f16, kind="ExternalOutput").ap()

    xT_r = xT.rearrange("(dt p) c -> p dt c", p=128)
    xp_r = xp.rearrange("(n p) d -> n p d", p=128)
    out_r = outd.rearrange("(n p) d -> n p d", p=128)

    def bcast_ap(src_ap, parts=128):
        return bass.AP(
            tensor=src_ap.tensor,
            offset=src_ap.offset,
            ap=[[0, parts]] + [list(x) for x in src_ap.ap],
        )

    with tile.TileContext(nc) as tc, ExitStack() as ctx:
        # ---- pools ----
        singles = ctx.enter_context(tc.tile_pool(name="singles", bufs=1))
        xt_pool = ctx.enter_context(tc.tile_pool(name="xt", bufs=2))
        kt_pool = ctx.enter_context(tc.tile_pool(name="kt", bufs=3))
        v_pool = ctx.enter_context(tc.tile_pool(name="v", bufs=2))
        qt_pool = ctx.enter_context(tc.tile_pool(name="qt", bufs=1))
        es_pool = ctx.enter_context(tc.tile_pool(name="es", bufs=2))
        at_pool = ctx.enter_context(tc.tile_pool(name="at", bufs=2))
        xp_pool = ctx.enter_context(tc.tile_pool(name="xpp", bufs=2))
        xhb_pool = ctx.enter_context(tc.tile_pool(name="xhb", bufs=5))
        xht_pool = ctx.enter_context(tc.tile_pool(name="xht", bufs=2))
        xhtl_pool = ctx.enter_context(tc.tile_pool(name="xhtl", bufs=2))
        ht_pool = ctx.enter_context(tc.tile_pool(name="ht", bufs=1))
        p2_pool = ctx.enter_context(tc.tile_pool(name="p2", bufs=4))
        xg_pool = ctx.enter_context(tc.tile_pool(name="xg", bufs=4))
        oh_pool = ctx.enter_context(tc.tile_pool(name="oh", bufs=2))
        wblk_pool = ctx.enter_context(tc.tile_pool(name="wblk", bufs=3))
        wv_pool = ctx.enter_context(tc.tile_pool(name="wv", bufs=1))
        w2_pool = ctx.enter_context(tc.tile_pool(name="w2", bufs=3))
        small = ctx.enter_context(tc.tile_pool(name="small", bufs=6))

        proj_ps = ctx.enter_context(tc.tile_pool(name="projps", bufs=3, space="PSUM"))
        qk_ps = ctx.enter_context(tc.tile_pool(name="qkps", bufs=3, space="PSUM"))
        avtp_ps = ctx.enter_context(tc.tile_pool(name="avtp", bufs=2, space="PSUM"))

        # ---- startup-critical DMAs first: first Q weight block + x slab ----
        wqb_first = wblk_pool.tile([128, 8, 128], f16, tag="wblk")
        nc.sync.dma_start(out=wqb_first, in_=wq[0])
        xt_first = xt_pool.tile([128, 8, CT], f16, tag="xt")
        nc.sync.dma_start(out=xt_first[:, 0:2, :], in_=xT_r[:, 0:2, HALO:HALO + CT])
        bq_sb = singles.tile([128, 8], f32)
        nc.sync.dma_start(out=bq_sb, in_=bqd)
        nc.sync.dma_start(out=xt_first[:, 2:8, :], in_=xT_r[:, 2:8, HALO:HALO + CT])
        xthalo = singles.tile([128, 8, HALO], f16)
        nc.sync.dma_start(out=xthalo, in_=xT_r[:, :, 0:HALO])

        # ---- constants: tiles now, DMAs deferred past chunk-0 weight loads
        ident = singles.tile([128, 128], f16)
        make_identity(nc, ident)
        cm = singles.tile([WIN, WIN], f16)
        m0t = singles.tile([WIN, WIN], f16)
        g1b = singles.tile([128, D], f16)
        g2b = singles.tile([128, D], f16)
        bt2b = singles.tile([128, D], f16)
        b2b = singles.tile([128, D], f16)
        b1_sb = singles.tile([128, 16], f32)

        def load_constants():
            nc.sync.dma_start(out=cm, in_=cmkd)
            nc.sync.dma_start(out=m0t, in_=m0d)
            nc.sync.dma_start(out=g1b, in_=bcast_ap(g1d))
            nc.sync.dma_start(out=g2b, in_=bcast_ap(g2d))
            nc.sync.dma_start(out=bt2b, in_=bcast_ap(bt2d))
            nc.sync.dma_start(out=b2b, in_=bcast_ap(b2d))
            nc.sync.dma_start(out=b1_sb, in_=b1d)

        epst = singles.tile([128, 1], f32)
        nc.vector.memset(epst, LN_EPS)
        ebt = singles.tile([128, 1], f32)
        nc.vector.memset(ebt, -6.931471805599453)
        kthalo = singles.tile([128, 8, 128], f16)
        v0_t = singles.tile([128, H, DH + 1], f16)

        cm_b = cm[:, :].unsqueeze(1).broadcast_to([128, 2, WIN])
        m0_b = m0t[:, :].unsqueeze(1).broadcast_to([128, 2, WIN])

        # ---- per-chunk state carried across the pipeline ----
        kt_prev = None       # kt slab of previous chunk
        v_prev = None        # v slab of previous chunk
        ht_prev = None       # FF1 output of previous chunk (filled this chunk)
        xht_prev = None      # x-hat^T of previous chunk (fp8 hi)
        xhtl_prev = None     # x-hat^T fp8 lo residual
        xhb_prev = None      # per-window xhb list of previous chunk
        xt_next = None       # prefetched xT slab

        def emit_qproj_unit(xt_t, qt_t, qc, wqb=None):
            if wqb is None:
                wqb = wblk_pool.tile([128, 8, 128], f16, tag="wblk", name="wqb")
                nc.sync.dma_start(out=wqb, in_=wq[qc])
            ps = proj_ps.tile([128, 512], f32, tag="pp")
            for d in range(8):
                nc.tensor.matmul(
                    ps, lhsT=wqb[:, d, :], rhs=xt_t[:, d, 0:CT],
                    start=(d == 0), stop=(d == 7),
                )
            nc.scalar.activation(
                qt_t[:, qc, :], ps, AF.Identity,
                bias=bq_sb[:, qc:qc + 1], scale=1.0,
            )

        def emit_kproj_unit(xt_t, kt_t, kc, first):
            wkb = wblk_pool.tile([128, 8, 128], f16, tag="wblk", name="wkb")
            nc.sync.dma_start(out=wkb, in_=wk[kc])
            ps = proj_ps.tile([128, 512], f32, tag="pp")
            for d in range(8):
                nc.tensor.matmul(
                    ps, lhsT=wkb[:, d, :], rhs=xt_t[:, d, 0:CT],
                    start=(d == 0), stop=(d == 7),
                )
            nc.scalar.activation(kt_t[:, kc, :], ps, AF.Copy, scale=1.0)
            if first:
                psh = proj_ps.tile([128, 512], f32, tag="pp")
                for d in range(8):
                    nc.tensor.matmul(
                        psh[:, 0:128], lhsT=wkb[:, d, :], rhs=xthalo[:, d, :],
                        start=(d == 0), stop=(d == 7),
                    )
                nc.scalar.activation(
                    kthalo[:, kc, :], psh[:, 0:128], AF.Copy, scale=1.0
                )

        def emit_vproj_half(xt_t, v_t, vc2, first, vts=None):
            wvb = wv_pool.tile([128, 8, 512], f16, tag="wv", name="wvb")
            nc.sync.dma_start(out=wvb, in_=wv[vc2])
            if vts is None:
                vts = range(-1, 4) if first else range(0, 4)
            for vt in vts:
                ps = proj_ps.tile([128, 512], f32, tag="pp")
                for d in range(8):
                    xsl = (xthalo[:, d, :] if vt < 0
                           else xt_t[:, d, vt * 128:(vt + 1) * 128])
                    nc.tensor.matmul(
                        ps, lhsT=xsl, rhs=wvb[:, d, :],
                        start=(d == 0), stop=(d == 7),
                    )
                psv = ps.rearrange("p (h e) -> p h e", e=DH)
                if vt < 0:
                    nc.scalar.activation(
                        v0_t[:, vc2 * 8:(vc2 + 1) * 8, 0:DH], psv,
                        AF.Copy, scale=1.0,
                    )
                else:
                    nc.scalar.activation(
                        v_t[:, vt, vc2 * 8:(vc2 + 1) * 8, 0:DH], psv,
                        AF.Copy, scale=1.0,
                    )

        def emit_proj(chn, xt_t, wqb0=None, kt_pre=None, v_pre=None):
            """Q/K/V projections for chunk chn. Returns (qt, kt, v)."""
            first = chn == 0
            qt_t = qt_pool.tile([128, 8, CT], f16, tag="qt")
            for qc in range(8):
                emit_qproj_unit(xt_t, qt_t, qc, wqb=wqb0 if qc == 0 else None)
            if kt_pre is not None:
                kt_t = kt_pre
            else:
                kt_t = kt_pool.tile([128, 8, CT], f16, tag="kt")
                for kc in range(8):
                    emit_kproj_unit(xt_t, kt_t, kc, first)
            if v_pre is not None:
                return qt_t, kt_t, v_pre
            v_t = v_pool.tile([128, 4, H, DH + 1], f16, tag="v")
            nc.vector.memset(v_t[:, :, :, DH:DH + 1], 1.0)
            if first:
                nc.vector.memset(v0_t[:, :, DH:DH + 1], 1.0)
            for vc2 in range(2):
                emit_vproj_half(xt_t, v_t, vc2, first)
            return qt_t, kt_t, v_t

        def kt_tile(chn, kt_t, j, h):
            """lhsT AP for k-tile j (0..4) of chunk chn, head h.

            Q/K features are host-permuted so head h sits at partition rows
            (h//8)*64.. and column tile h%8 — score groups of 4 consecutive
            heads then share one PE row base (mixed row bases into one PSUM
            tile crash the exec unit)."""
            r0 = (h // 8) * 64
            if j == 0:
                if chn == 0:
                    return kthalo[r0:r0 + 64, h % 8, :]
                return kt_prev[r0:r0 + 64, h % 8, 384:512]
            return kt_t[r0:r0 + 64, h % 8, (j - 1) * 128:j * 128]

        def v_tile(chn, v_t, j, h):
            if j == 0:
                if chn == 0:
                    return v0_t[:, h, :]
                return v_prev[:, 3, h, :]
            return v_t[:, j - 1, h, :]

        def emit_ff1_quarter(q):
            """FF1 for hc in [4q, 4q+4): split-fp8 DoubleRow (hi*hi +
            hi*lo + lo*hi), 25% fewer PE cycles than bf16."""
            for hc in range(4 * q, 4 * q + 4):
                w1b = wblk_pool.tile([128, 16, 128], f8, tag="wblk", name="w1b")
                nc.sync.dma_start(out=w1b, in_=w1hl[hc])
                ps = proj_ps.tile([128, 512], f32, tag="pp")
                n = 0
                for (lo, R) in ((0, xht_prev), (0, xhtl_prev),
                                (8, xht_prev)):
                    for i in range(4):
                        nc.tensor.matmul(
                            ps, lhsT=w1b[:, lo + 2 * i:lo + 2 * i + 2, :],
                            rhs=R[:, 2 * i:2 * i + 2, :],
                            start=(n == 0), stop=(n == 11), perf_mode=DR,
                        )
                        n += 1
                if simple_ln:
                    nc.scalar.activation(ht_prev[:, hc, :], ps, AF.Relu,
                                         scale=1.0)
                    nc.vector.scalar_tensor_tensor(
                        htl_prev[:, hc, :], ps, 0.0, ht_prev[:, hc, :],
                        op0=ALU.max, op1=ALU.subtract,
                    )
                else:
                    nc.scalar.activation(
                        ht_prev[:, hc, :], ps, AF.Relu,
                        bias=b1_sb[:, hc:hc + 1], scale=1.0,
                    )

        def make_xg(xhb_t):
            xg_t = xg_pool.tile([128, D], f16, tag="xg", name="xg_t")
            if simple_ln:
                nc.vector.tensor_add(xg_t, xhb_t, b2b)
            else:
                nc.vector.tensor_mul(xg_t, xhb_t, g1b)
                nc.vector.tensor_add(xg_t, xg_t, b2b)
            return xg_t

        def emit_ff2(wg_base, xg_list, last=False):
            """FF2 + LN2 + output for the previous chunk (its global window
            base is wg_base)."""
            p2_list = [
                p2_pool.tile([128, D], f32, tag="p2", name=f"p2_{t}")
                for t in range(CW)
            ]
            st2_list = [
                small.tile([128, 8, 6], f32, tag="st2", name=f"st2_{t}")
                for t in range(CW)
            ]
            def emit_ln2(t):
                mv2 = small.tile([128, 2], f32, tag="mv2")
                nc.vector.bn_aggr(out=mv2, in_=st2_list[t])
                lv2 = small.tile([128, 1], f32, tag="sd2")
                nc.scalar.activation(lv2, mv2[:, 1:2], AF.Ln, bias=epst)
                rstd2 = small.tile([128, 1], f32, tag="rstd2")
                nc.scalar.activation(rstd2, lv2, AF.Exp, scale=-0.5)
                nmr2 = small.tile([128, 1], f32, tag="nmr2")
                nc.vector.tensor_scalar(
                    nmr2, mv2[:, 0:1], rstd2, -1.0, op0=ALU.mult, op1=ALU.mult
                )
                oh = oh_pool.tile([128, D], f16, tag="oh", name="oh")
                for yc in range(2):
                    sl2 = slice(yc * 512, (yc + 1) * 512)
                    nc.scalar.activation(
                        oh[:, sl2], p2_list[t][:, sl2], AF.Identity,
                        bias=nmr2, scale=rstd2,
                    )
                    if not simple_ln:
                        nc.vector.tensor_mul(
                            oh[:, sl2], oh[:, sl2], g2b[:, sl2])
                        nc.vector.tensor_add(
                            oh[:, sl2], oh[:, sl2], bt2b[:, sl2])
                    nc.sync.dma_start(
                        out=out_r[wg_base + t][:, sl2], in_=oh[:, sl2]
                    )

            for y8 in range(8):
                if simple_ln:
                    w2b = w2_pool.tile([128, 32, 128], f8, tag="w2")
                    nc.sync.dma_start(out=w2b, in_=w2hl[y8])
                else:
                    w2b = w2_pool.tile([128, 16, 128], f16, tag="w2")
                    nc.sync.dma_start(out=w2b, in_=w2[y8])
                psp = qk_ps if y8 % 2 else proj_ps
                ps = psp.tile([128, 512], f32, tag="qk" if y8 % 2 else "pp")
                for t in range(CW):
                    sl = slice(t * 128, (t + 1) * 128)
                    if simple_ln:
                        n = 0
                        for (L, ro) in ((ht_prev, 0), (ht_prev, 16),
                                        (htl_prev, 0)):
                            for i in range(8):
                                nc.tensor.matmul(
                                    ps[:, sl],
                                    lhsT=L[:, 2 * i:2 * i + 2, sl],
                                    rhs=w2b[:, ro + 2 * i:ro + 2 * i + 2, :],
                                    start=(n == 0), stop=(n == 23),
                                    perf_mode=DR,
                                )
                                n += 1
                    else:
                        for hc in range(16):
                            nc.tensor.matmul(
                                ps[:, sl], lhsT=ht_prev[:, hc, sl],
                                rhs=w2b[:, hc, :],
                                start=(hc == 0), stop=(hc == 15),
                            )
                    ysl = slice(y8 * 128, (y8 + 1) * 128)
                    nc.vector.tensor_add(
                        p2_list[t][:, ysl], ps[:, sl], xg_list[t][:, ysl]
                    )
                    nc.vector.bn_stats(
                        out=st2_list[t][:, y8, :], in_=p2_list[t][:, ysl]
                    )
                    if y8 == 7:
                        emit_ln2(t)

        def emit_tp(w, xhb_w, xht_t, xhtl_t, on_act=False):
            """Transpose window w's x-hat into fp8 hi/lo xht slabs."""
            tp = avtp_ps.tile([128, 8, 128], f16, tag="avtp")
            for dt in range(8):
                nc.tensor.transpose(
                    tp[:, dt, :], xhb_w[:, dt * 128:(dt + 1) * 128], ident
                )
            wsl = slice(w * 128, (w + 1) * 128)
            nc.scalar.activation(xht_t[:, :, wsl], tp, AF.Copy, scale=1.0)
            nc.vector.tensor_sub(xhtl_t[:, :, wsl], tp, xht_t[:, :, wsl])

        xt_next = xt_first

        next_proj = None     # (qt, kt, v) pre-emitted for this chunk
        kt_next = None       # K slab of the next chunk, filled in our windows
        for chn in range(NCH):
            xt_t = xt_next
            if next_proj is not None:
                qt_t, kt_t, v_t = next_proj
                next_proj = None
            else:
                qt_t, kt_t, v_t = emit_proj(
                    chn, xt_t,
                    wqb0=wqb_first if chn == 0 else wqb_next,
                    kt_pre=kt_next)
            if 0 < chn < NCH - 1:
                kt_next = kt_pool.tile([128, 8, CT], f16, tag="kt",
                                       name="kt_next")
            else:
                kt_next = None
            if chn == 0:
                load_constants()
            if chn + 1 < NCH:
                xt_next = xt_pool.tile([128, 8, CT], f16, tag="xt")
                c0 = (chn + 1) * CT + HALO
                nc.sync.dma_start(out=xt_next, in_=xT_r[:, :, c0:c0 + CT])
            if chn == 0:
                kt1 = kt_pool.tile([128, 8, CT], f16, tag="kt")
                qt1 = qt_pool.tile([128, 8, CT], f16, tag="qt")
                v1 = v_pool.tile([128, 4, H, DH + 1], f16, tag="v")
                nc.vector.memset(v1[:, :, :, DH:DH + 1], 1.0)

            xht_t = xht_pool.tile([128, 8, CT], f8, tag="xht")
            xhtl_t = xhtl_pool.tile([128, 8, CT], f8, tag="xhtl")
            xhb_list = []
            xg_cur = []
            if chn > 0:
                if simple_ln:
                    ht_prev = ht_pool.tile([128, 16, CT], f8, tag="ht")
                    htl_prev = ht_pool.tile([128, 16, CT], f8, tag="htl")
                else:
                    ht_prev = ht_pool.tile([128, 16, CT], f16, tag="ht")

            for w in range(CW):
                wg = chn * CW + w
                # residual slab for this window (early DMA)
                xp_t = xp_pool.tile([128, D], f16, tag="xp")
                nc.sync.dma_start(out=xp_t, in_=xp_r[wg])

                # ---- scores: QK^T, exp + mask in 4-head groups (pipelined)
                es = es_pool.tile([128, H, 2 * WIN], f16, tag="es")
                for g in range(8):
                    st = qk_ps.tile([128, 2, 256], f32, tag="qk")
                    for hh in range(2):
                        h = g * 2 + hh
                        r0 = (h // 8) * 64
                        for half in range(2):
                            j = w + half
                            nc.tensor.matmul(
                                st[:, hh, half * 128:(half + 1) * 128],
                                lhsT=kt_tile(chn, kt_t, j, h),
                                rhs=qt_t[r0:r0 + 64, h % 8, w * 128:(w + 1) * 128],
                                start=True, stop=True,
                            )
                    gs = slice(g * 2, (g + 1) * 2)
                    nc.scalar.activation(es[:, gs, :], st, AF.Exp, bias=ebt)
                    nc.vector.tensor_mul(
                        es[:, gs, WIN:2 * WIN], es[:, gs, WIN:2 * WIN], cm_b
                    )
                    if wg == 0:
                        nc.vector.tensor_mul(
                            es[:, gs, 0:WIN], es[:, gs, 0:WIN], m0_b
                        )

                # ---- PE fillers while softmax chain runs
                if w > 0:
                    emit_tp(w - 1, xhb_list[w - 1], xht_t, xhtl_t)
                if chn > 0:
                    emit_ff1_quarter(w)
                    if kt_next is not None:
                        emit_kproj_unit(xt_next, kt_next, 2 * w, False)
                        emit_kproj_unit(xt_next, kt_next, 2 * w + 1, False)
                else:
                    emit_kproj_unit(xt_next, kt1, 2 * w, False)
                    emit_kproj_unit(xt_next, kt1, 2 * w + 1, False)
                    emit_vproj_half(xt_next, v1, w // 2, False,
                                    vts=(2 * (w % 2), 2 * (w % 2) + 1))

                # ---- AV in 4-head groups with ones-column denominators
                at_t = at_pool.tile([128, D], f32, tag="at")
                at_r = at_t.rearrange("p (h e) -> p h e", e=DH)
                rden = small.tile([128, H], f32, tag="rden")
                for g in range(4):
                    av = avtp_ps.tile([128, 512], f32, tag="avtp")
                    av_r = av.rearrange("p (h x) -> p h x", x=128)
                    for hh in range(4):
                        h = g * 4 + hh
                        for half in range(2):
                            j = w + half
                            nc.tensor.matmul(
                                av[:, hh * 128:hh * 128 + DH + 1],
                                lhsT=es[:, h, half * 128:(half + 1) * 128],
                                rhs=v_tile(chn, v_t, j, h),
                                start=(half == 0), stop=(half == 1),
                            )
                    nc.vector.reciprocal(
                        rden[:, g * 4:(g + 1) * 4], av_r[:, :, DH:DH + 1]
                    )
                    nc.vector.tensor_mul(
                        at_r[:, g * 4:(g + 1) * 4, :],
                        av_r[:, :, 0:DH],
                        rden[:, g * 4:(g + 1) * 4].unsqueeze(2).broadcast_to(
                            [128, 4, DH]
                        ),
                    )

                # ---- residual + LN1
                nc.vector.tensor_add(at_t, at_t, xp_t)
                stats = small.tile([128, 2, 6], f32, tag="stats")
                pre1v = at_t.rearrange("p (a b) -> p a b", b=512)
                for sg in range(2):
                    nc.vector.bn_stats(out=stats[:, sg, :], in_=pre1v[:, sg, :])
                mv = small.tile([128, 2], f32, tag="mv")
                nc.vector.bn_aggr(out=mv, in_=stats)
                lv = small.tile([128, 1], f32, tag="sd")
                nc.scalar.activation(lv, mv[:, 1:2], AF.Ln, bias=epst)
                rstd = small.tile([128, 1], f32, tag="rstd")
                nc.scalar.activation(rstd, lv, AF.Exp, scale=-0.5)
                nmr = small.tile([128, 1], f32, tag="nmr")
                nc.vector.tensor_scalar(
                    nmr, mv[:, 0:1], rstd, -1.0, op0=ALU.mult, op1=ALU.mult
                )
                xhb = xhb_pool.tile([128, D], f16, tag="xhb")
                nc.scalar.activation(xhb, at_t, AF.Identity, bias=nmr, scale=rstd)
                xhb_list.append(xhb)
                if chn > 0:
                    xg_cur.append(make_xg(xhb_prev[w]))

            # ---- chunk tail: last transpose, then FF2 of previous chunk
            if chn == 0:
                emit_qproj_unit(xt_next, qt1, 0)
                emit_qproj_unit(xt_next, qt1, 1)
                emit_tp(CW - 1, xhb_list[CW - 1], xht_t, xhtl_t, on_act=True)
                for qc in range(2, 8):
                    emit_qproj_unit(xt_next, qt1, qc)
                next_proj = (qt1, kt1, v1)
            else:
                emit_tp(CW - 1, xhb_list[CW - 1], xht_t, xhtl_t, on_act=True)
            if chn > 0:
                if chn + 1 < NCH:
                    wqb_next = wblk_pool.tile(
                        [128, 8, 128], f16, tag="wblk", name="wqb_next")
                    nc.sync.dma_start(out=wqb_next, in_=wq[0])
                emit_ff2((chn - 1) * CW, xg_cur)

            kt_prev, v_prev = kt_t, v_t
            xht_prev, xhtl_prev = xht_t, xhtl_t
            xhb_prev = xhb_list

        # ---- epilogue: FF1 + FF2 of the last chunk ----
        if simple_ln:
            ht_prev = ht_pool.tile([128, 16, CT], f8, tag="ht")
            htl_prev = ht_pool.tile([128, 16, CT], f8, tag="htl")
        else:
            ht_prev = ht_pool.tile([128, 16, CT], f16, tag="ht")
        xg_last = [make_xg(xhb_prev[t]) for t in range(CW)]
        for q in range(CW):
            emit_ff1_quarter(q)
        emit_ff2((NCH - 1) * CW, xg_last, last=True)

    # Steer every activation to the one table containing Exp+Ln+Identity+
    # Copy+Relu ('natural_log_exp_and_others') so a single LoadActFuncSet
    # suffices; the greedy placement otherwise alternates exp/ln tables at
    # 1.28us per reload. Names and dict order (= act_func_set_id) are kept.
    import concourse.bacc as bacc_mod
    orig_tables = bacc_mod.get_activation_tables
    target = "natural_log_exp_and_others"
    mine = {AF.Exp, AF.Ln, AF.Identity, AF.Copy, AF.Relu}

    def steered(arch):
        tabs = orig_tables(arch)
        return {
            name: (set(s) if name == target else set(s) - mine)
            for name, s in tabs.items()
        }

    bacc_mod.get_activation_tables = steered
    try:
        nc.compile()
    finally:
        bacc_mod.get_activation_tables = orig_tables
    return nc


def _is_fast(ln1_g, ln2_g, ln2_b, ff1_b, ln1_b):
    return (np.allclose(np.asarray(ln2_g, np.float32), 1.0)
            and np.allclose(np.asarray(ln2_b, np.float32), 0.0)
            and np.allclose(np.asarray(ln1_g, np.float32), 1.0)
            and np.allclose(np.asarray(ff1_b, np.float32), 0.0)
            and np.allclose(np.asarray(ln1_b, np.float32), 0.0))


def _get_program(simple_ln=False):
    key = ("nc", simple_ln)
    if key not in _PROGRAM_CACHE:
        _PROGRAM_CACHE[key] = _build_program(simple_ln=simple_ln)
    return _PROGRAM_CACHE[key]


def make_in_maps(x, q_proj, k_proj, v_proj, q_bias, k_bias, v_bias,
                 ln1_g, ln1_b, ln2_g, ln2_b, ff1_w, ff1_b, ff2_w, ff2_b):
    """Host-side prep: fold biases/scales, relayout weights, shard."""
    x = np.asarray(x, np.float32)
    scale = DH ** -0.5

    Wq = (np.transpose(np.asarray(q_proj, np.float32), (1, 0, 2)).reshape(D, D)
          * scale)
    Wk = np.transpose(np.asarray(k_proj, np.float32), (1, 0, 2)).reshape(D, D)
    Wv = np.transpose(np.asarray(v_proj, np.float32), (1, 0, 2)).reshape(D, D)
    bq_full = (np.asarray(q_bias, np.float32).reshape(D) * scale)
    bv_full = np.asarray(v_bias, np.float32).reshape(D)

    ln1_g = np.asarray(ln1_g, np.float32)
    ln1_b = np.asarray(ln1_b, np.float32)
    ff1_w = np.asarray(ff1_w, np.float32)
    ff1_b = np.asarray(ff1_b, np.float32)
    ff2_w = np.asarray(ff2_w, np.float32)
    ff2_b = np.asarray(ff2_b, np.float32)

    W1 = ff1_w * ln1_g[:, None]                 # fold ln1_g into rows
    b1_full = ff1_b + ln1_b @ ff1_w             # fold ln1_b into ff1 bias
    b2_full = ff2_b + ln1_b                     # fold ln1_b into ff2 bias

    # Q/K head permutation: head h -> partition rows (h//8)*64, col tile h%8,
    # so 4-consecutive-head score groups share one PE row base.
    qk_perm = np.empty(D, np.int64)
    for h in range(H):
        e = np.arange(DH)
        qk_perm[(h % 8) * 128 + (h // 8) * 64 + e] = h * DH + e
    Wq = Wq[:, qk_perm]
    Wk = Wk[:, qk_perm]
    bq_full = bq_full[qk_perm]

    def relayout(W, n_out_blk, blk):
        # [K, N] -> [nb, 128, K//128, blk] so each SBUF partition's data is
        # one contiguous run per DMA block.
        K, N = W.shape
        a = W.reshape(K // 128, 128, n_out_blk, blk).transpose(2, 1, 0, 3)
        return np.ascontiguousarray(a).astype(np.float16)

    wq_l = relayout(Wq, 8, 128)
    wk_l = relayout(Wk, 8, 128)
    wv_l = relayout(Wv, 2, 512)
    E4 = ml_dtypes.float8_e4m3

    def relayout8(W, n_out_blk, blk):
        K, N = W.shape
        a = W.reshape(K // 128, 128, n_out_blk, blk).transpose(2, 1, 0, 3)
        return np.ascontiguousarray(a).astype(E4)

    W1h = W1.astype(E4).astype(np.float32)
    w1hl_l = np.concatenate(
        [relayout8(W1h, 16, 128), relayout8(W1 - W1h, 16, 128)], axis=2)
    if _is_fast(ln1_g, ln2_g, ln2_b, ff1_b, ln1_b):
        W2h = ff2_w.astype(E4).astype(np.float32)
        w2_kv = {"w2hl": np.concatenate(
            [relayout8(W2h, 8, 128), relayout8(ff2_w - W2h, 8, 128)], axis=2)}
    else:
        w2_kv = {"w2": relayout(ff2_w, 8, 128)}

    bq_l = np.ascontiguousarray(bq_full.reshape(8, 128).T.astype(np.float32))
    b1_l = np.ascontiguousarray(b1_full.reshape(16, 128).T.astype(np.float32))

    kq = np.arange(WIN)
    cmk = (kq[None, :] >= kq[:, None]).astype(np.float16)  # [k, q]

    common = {
        "wq": wq_l, "wk": wk_l, "wv": wv_l,
        "bq": bq_l, "w1hl": w1hl_l, "b1": b1_l, **w2_kv,
        "b2": b2_full.astype(np.float16),
        "g1": ln1_g.astype(np.float16),
        "g2": np.asarray(ln2_g, np.float16),
        "bt2": np.asarray(ln2_b, np.float16),
        "cmk": cmk,
    }

    in_maps = []
    for b in range(B):
        for s in range(2):
            own = x[b, s * T:(s + 1) * T]
            if s == 0:
                halo = np.zeros((HALO, D), np.float32)
                m0 = np.zeros((WIN, WIN), np.float16)
            else:
                halo = x[b, s * T - HALO:s * T]
                m0 = np.ones((WIN, WIN), np.float16)
            xta = np.ascontiguousarray(
                np.concatenate([halo, own], axis=0).T).astype(np.float16)
            xpa = own + bv_full[None, :]
            in_maps.append({
                **common,
                "xT": xta,
                "xp": np.ascontiguousarray(xpa).astype(np.float16),
                "m0": m0,
            })
    return in_maps


def gather_outputs(results):
    out = np.empty((B, S, D), np.float32)
    for b in range(B):
        for s in range(2):
            out[b, s * T:(s + 1) * T] = np.asarray(
                results[b * 2 + s]["out"], dtype=np.float32)
    return out


def kernel(**inputs):
    from concourse import bass_utils

    simple_ln = _is_fast(inputs["ln1_g"], inputs["ln2_g"],
                         inputs["ln2_b"], inputs["ff1_b"], inputs["ln1_b"])
    nc = _get_program(simple_ln=simple_ln)
    in_maps = make_in_maps(**inputs)
    res = bass_utils.run_bass_kernel_spmd(nc, in_maps, core_ids=list(range(8)))
    return gather_outputs(res.results)
